# revision 14
# baseline (speedup 1.0000x reference)
"""Single-head attention (B=4, S=2048, D=1024) on 8 TRN2 NeuronCores.

Sharding: data-parallel over (batch, sequence-half) — core i owns the
1024 queries of block (i//2, i%2); no collectives. Host-side work is
layout/weight-space only (transposes, fp16/bf16 casts, and fp64
weight folding); all x-dependent compute runs on the NeuronCores.

Algorithmic restructure (exact up to rounding):
  scores = (xWq+bq)(xWk+bk)^T = x A x^T + 1.w^T (+ per-row consts that
  cancel in softmax), with A = Wq Wk^T and w = x.(Wk bq); A and Wk bq
  are folded on the host in fp64. The value path is reassociated as
  out = P(x Wv + bv)/colsum = ((P x) Wv)/colsum + bv, which removes the
  full-sequence V projection entirely.

On-chip phases (per core, all matmul tiles 128x128x512):
  MT'[d,q] = A^T-contraction of xT + b2 bias   (128 MMs, fp16)
  scoresT[k,q] = xT-stationary contraction with MT'  (256 MMs, fp16)
  softmax along k (partitions): exp on ScalarE -> P (bf16, no max
    subtraction needed: |scores| < 80 fits fp32), column sums via
    ones-vector matmuls (32 MMs), reciprocal on VectorE, broadcast to
    128 partitions via an fp32 outer-product matmul
  Y^T[d,q] = x-natural-stationary contraction with P^T (256 MMs, bf16)
  outT[e,q] = Wv-stationary contraction with Y^T (128 MMs, bf16),
    normalized + bv added on the VectorE epilogue, bf16 out, host
    transposes/upcasts.

Perf notes (measured): both HW DMA queues (sync+scalar) are in-order —
loads are issued in first-use order with A pre-tiled on the host so
every DMA is contiguous; a dummy Exp preloads the ScalarE activation
table; dummy matmuls on memset tiles warm the PE clock gate (HAM)
during the initial DMA wait. ~205 us on silicon, rel err ~4e-3
(PE occupancy ~90%, steady-state matmul cadence at the 512-cycle
roofline).
"""

import os

import numpy as np
import ml_dtypes

import concourse.bass as bass
from concourse import bacc
import concourse.mybir as mybir
import concourse.tile as tile
from concourse.bass_utils import run_bass_kernel_spmd

BF = mybir.dt.bfloat16
F16 = mybir.dt.float16
F32 = mybir.dt.float32

B, S, D = 4, 2048, 1024
SQ = S // 2          # queries per core
NDT = D // 128       # 8 d-tiles
NKT = S // 128       # 16 k-tiles
NQC = SQ // 512      # 2 query chunks of 512
NEC = D // 512       # 2 feature chunks of 512

LAST_EXEC_TIME_NS = None
LAST_TRACE = None


def _build():
    nc = bacc.Bacc(None)

    xt_ext = nc.declare_dram_parameter("xt", [D, S], F16, isOutput=False)
    xn_ext = nc.declare_dram_parameter("xn", [S, D], BF, isOutput=False)
    wa_ext = nc.declare_dram_parameter("wa", [D, D], F16, isOutput=False)
    wv_ext = nc.declare_dram_parameter("wv", [D, D], BF, isOutput=False)
    b2_ext = nc.declare_dram_parameter("b2", [D], F32, isOutput=False)
    bv_ext = nc.declare_dram_parameter("bv", [D], F32, isOutput=False)
    out_ext = nc.declare_dram_parameter("out", [D, SQ], BF, isOutput=True)

    with tile.TileContext(nc) as tc:
        with (
            tc.tile_pool(name="xt", bufs=NDT) as xt_pool,
            tc.tile_pool(name="wst", bufs=NDT) as wst_pool,
            tc.tile_pool(name="wv", bufs=NDT) as wv_pool,
            tc.tile_pool(name="mt", bufs=NDT) as mt_pool,
            tc.tile_pool(name="xn", bufs=NKT) as xn_pool,
            tc.tile_pool(name="yt", bufs=NDT) as yt_pool,
            tc.tile_pool(name="pt", bufs=NKT) as pt_pool,
            tc.tile_pool(name="small", bufs=1) as small,
            tc.tile_pool(name="ot", bufs=4) as ot_pool,
            tc.tile_pool(name="ps", bufs=6, space="PSUM") as ps_pool,
            tc.tile_pool(name="pcs", bufs=2, space="PSUM") as pcs_pool,
        ):
            # ---- input loads. Two in-order HW DMA queues (sync + scalar);
            # order within each queue follows first-use. wa arrives host-
            # pre-tiled so every load is a fully contiguous row-block.
            wa_sb = [
                wst_pool.tile([128, D], F16, tag="wst", name="wa")
                for _ in range(NDT)
            ]
            xt = [xt_pool.tile([128, S], F16, tag="xt", name="xt") for _ in range(NDT)]
            for dt in range(4):
                nc.sync.dma_start(out=xt[dt], in_=xt_ext[dt * 128 : (dt + 1) * 128, :])
            for dto in range(3):
                nc.scalar.dma_start(
                    out=wa_sb[dto],
                    in_=wa_ext[dto * 128 : (dto + 1) * 128, :],
                )
            for dt in range(4, NDT):
                nc.scalar.dma_start(out=xt[dt], in_=xt_ext[dt * 128 : (dt + 1) * 128, :])
            for dto in range(3, NDT):
                nc.scalar.dma_start(
                    out=wa_sb[dto],
                    in_=wa_ext[dto * 128 : (dto + 1) * 128, :],
                )
            b2_sb = small.tile([128, NDT], F32, tag="b2")
            nc.sync.dma_start(out=b2_sb, in_=b2_ext[:].rearrange("(e p) -> p e", p=128))
            ones_col = small.tile([128, 1], BF, tag="onc")
            nc.vector.memset(ones_col, 1.0)
            # dummy exp: forces the ScalarE activation table load at t=0,
            # so it is not queued behind the bulk input DMAs later
            exp_warm = small.tile([128, 1], F32, tag="expw")
            nc.scalar.activation(
                out=exp_warm, in_=ones_col,
                func=mybir.ActivationFunctionType.Exp,
            )
            # HAM warm-up: dense dummy matmuls on a memset tile while the
            # first input DMAs are in flight, so the PE clock gate reaches
            # 8/8 before real work starts (and the first-data wait is not
            # PE-idle time).
            warm_rhs = small.tile([128, 512], F16, tag="warmr")
            nc.vector.memset(warm_rhs, 0.0)
            ones_f16 = small.tile([128, 1], F16, tag="on16")
            nc.vector.memset(ones_f16, 1.0)
            warm_ps = pcs_pool.tile([1, 512], F32, tag="cs", name="warmps")
            for _ in range(12):
                nc.tensor.matmul(warm_ps, lhsT=ones_f16, rhs=warm_rhs,
                                 start=True, stop=True)
            ones_row_f = small.tile([1, 128], F32, tag="onrf")
            nc.vector.memset(ones_row_f, 1.0)

            # ---- MT'[d,q] = (A^T-contraction of xT) + b2[d], queries only.
            # dp-major waves so PE consumes xT tiles as their DMAs land.
            # The per-partition b2 bias on the PSUM copy makes the later
            # scoresT contraction produce scores + w[k] exactly (w = x.b2).
            mt_sb = [None] * NDT
            for wave in ((0, 1, 2), (3, 4, 5), (6, 7)):
                accs = {}
                for dto in wave:
                    for q in range(NQC):
                        accs[dto, q] = ps_pool.tile(
                            [128, 512], F32, tag="acc", name="acc"
                        )
                for dp in range(NDT):
                    for dto in wave:
                        for q in range(NQC):
                            nc.tensor.matmul(
                                accs[dto, q],
                                lhsT=wa_sb[dto][:, dp * 128 : (dp + 1) * 128],
                                rhs=xt[dp][:, q * 512 : (q + 1) * 512],
                                start=(dp == 0),
                                stop=(dp == NDT - 1),
                            )
                for dto in wave:
                    mt_t = mt_pool.tile([128, SQ], F16, tag="mt")
                    for q in range(NQC):
                        nc.vector.tensor_scalar_add(
                            out=mt_t[:, q * 512 : (q + 1) * 512],
                            in0=accs[dto, q],
                            scalar1=b2_sb[:, dto : dto + 1],
                        )
                    mt_sb[dto] = mt_t

            # deferred loads: needed from the Y^T / output phases onward
            xn = []
            for st in range(NKT):
                t = xn_pool.tile([128, D], BF, tag="xn")
                nc.scalar.dma_start(out=t, in_=xn_ext[st * 128 : (st + 1) * 128, :])
                xn.append(t)
            wv_sb = []
            for dt in range(NDT):
                t = wv_pool.tile([128, D], BF, tag="wv")
                nc.scalar.dma_start(out=t, in_=wv_ext[dt * 128 : (dt + 1) * 128, :])
                wv_sb.append(t)
            bv_sb = small.tile([128, NDT], F32, tag="bv")
            nc.sync.dma_start(out=bv_sb, in_=bv_ext[:].rearrange("(e p) -> p e", p=128))

            # ---- scoresT[k,q] (w[k] folded into MT via b2 bias) + exp ----
            cs_acc = [pcs_pool.tile([1, 512], F32, tag="cs", name="cs") for _ in range(NQC)]
            pt_sb = []
            for kt in range(NKT):
                acc = [ps_pool.tile([128, 512], F32, tag="acc", name="acc") for _ in range(NQC)]
                for dp in range(NDT):
                    for q in range(NQC):
                        nc.tensor.matmul(
                            acc[q],
                            lhsT=xt[dp][:, kt * 128 : (kt + 1) * 128],
                            rhs=mt_sb[dp][:, q * 512 : (q + 1) * 512],
                            start=(dp == 0),
                            stop=(dp == NDT - 1),
                        )
                pt_t = pt_pool.tile([128, SQ], BF, tag="pt")
                for q in range(NQC):
                    nc.scalar.activation(
                        out=pt_t[:, q * 512 : (q + 1) * 512],
                        in_=acc[q],
                        func=mybir.ActivationFunctionType.Exp,
                    )
                pt_sb.append(pt_t)

            # column sums emitted after the scores loop: by then every
            # exp output is long since ready, so these matmuls carry no
            # cross-engine waits (in-loop placement stalled PE ~190ns/kt)
            for kt in range(NKT):
                for q in range(NQC):
                    nc.tensor.matmul(
                        cs_acc[q],
                        lhsT=ones_col,
                        rhs=pt_sb[kt][:, q * 512 : (q + 1) * 512],
                        start=(kt == 0),
                        stop=(kt == NKT - 1),
                    )

            # ---- softmax denominator ----
            cs_sb = small.tile([1, SQ], F32, tag="css")
            for q in range(NQC):
                nc.vector.tensor_copy(
                    out=cs_sb[:, q * 512 : (q + 1) * 512], in_=cs_acc[q]
                )
            rc_sb = small.tile([1, SQ], F32, tag="rcs")
            nc.vector.reciprocal(out=rc_sb, in_=cs_sb)

            # ---- Y^T[d,q] = sum_k x[k,d] P^T[k,q], normalized on the
            # PSUM copy (1/colsum commutes through the Wv contraction, so
            # the output epilogue needs no VectorE work at all) ----
            yt_sb = []
            bc_sb = small.tile([128, SQ], F32, tag="bcs")
            for dto in range(NDT):
                acc = [ps_pool.tile([128, 512], F32, tag="acc", name="acc") for _ in range(NQC)]
                for kt in range(NKT):
                    for q in range(NQC):
                        nc.tensor.matmul(
                            acc[q],
                            lhsT=xn[kt][:, dto * 128 : (dto + 1) * 128],
                            rhs=pt_sb[kt][:, q * 512 : (q + 1) * 512],
                            start=(kt == 0),
                            stop=(kt == NKT - 1),
                        )
                if dto == 0:
                    # broadcast 1/colsum to 128 partitions now: the DVE
                    # recip chain finished during group 0's matmuls, so
                    # these PE outer-products carry no waits
                    for q in range(NQC):
                        bc_ps = pcs_pool.tile([128, 512], F32, tag="cs", name="bc")
                        nc.tensor.matmul(
                            bc_ps,
                            lhsT=ones_row_f,
                            rhs=rc_sb[:, q * 512 : (q + 1) * 512],
                            start=True,
                            stop=True,
                        )
                        nc.vector.tensor_copy(
                            out=bc_sb[:, q * 512 : (q + 1) * 512], in_=bc_ps
                        )
                yt_t = yt_pool.tile([128, SQ], BF, tag="yt")
                for q in range(NQC):
                    nc.vector.tensor_mul(
                        out=yt_t[:, q * 512 : (q + 1) * 512],
                        in0=acc[q],
                        in1=bc_sb[:, q * 512 : (q + 1) * 512],
                    )
                yt_sb.append(yt_t)

            # ---- outT[e,q] = Wv.T @ Y^T + colsum x bv, normalized ----
            for et in range(NDT):
                acc = [ps_pool.tile([128, 512], F32, tag="acc", name="acc") for _ in range(NQC)]
                for dt in range(NDT):
                    for q in range(NQC):
                        nc.tensor.matmul(
                            acc[q],
                            lhsT=wv_sb[dt][:, et * 128 : (et + 1) * 128],
                            rhs=yt_sb[dt][:, q * 512 : (q + 1) * 512],
                            start=(dt == 0),
                            stop=(dt == NDT - 1),
                        )
                for q in range(NQC):
                    ot_t = ot_pool.tile([128, 512], BF, tag="ot")
                    nc.scalar.activation(
                        out=ot_t,
                        in_=acc[q],
                        func=mybir.ActivationFunctionType.Identity,
                        bias=bv_sb[:, et : et + 1],
                    )
                    nc.sync.dma_start(
                        out=out_ext[
                            et * 128 : (et + 1) * 128, q * 512 : (q + 1) * 512
                        ],
                        in_=ot_t,
                    )
    nc.finalize()
    return nc


def _install_trace_shims():
    import sys
    import types

    if "antenv.axon_hooks" not in sys.modules:
        import antenv

        mod = types.ModuleType("antenv.axon_hooks")
        mod._hook = None

        def set_axon_ntff_profile_hook(h):
            mod._hook = h

        def get_axon_ntff_profile_hook():
            return mod._hook

        mod.set_axon_ntff_profile_hook = set_axon_ntff_profile_hook
        mod.get_axon_ntff_profile_hook = get_axon_ntff_profile_hook
        sys.modules["antenv.axon_hooks"] = mod
        antenv.axon_hooks = mod
        try:
            from trn_agent_boot.trn_boot import _ntff_profile_via_ctypes

            hook = _ntff_profile_via_ctypes("/opt/axon/libaxon_pjrt.so")
            if hook is not None:
                set_axon_ntff_profile_hook(hook)
        except Exception:
            pass
    from concourse import bass_utils as bu

    bu.upload_artifacts = lambda tmpdir: tmpdir


def _bf16(a):
    return np.ascontiguousarray(a).astype(ml_dtypes.bfloat16)


def _f16(a):
    return np.ascontiguousarray(a).astype(np.float16)


def kernel(x, Wq, bq, Wk, bk, Wv, bv):
    global LAST_EXEC_TIME_NS, LAST_TRACE
    x = np.asarray(x, dtype=np.float32)
    Wq64 = np.asarray(Wq, np.float64)
    Wk64 = np.asarray(Wk, np.float64)
    A = Wq64 @ Wk64.T                      # [D, D] fused QK^T weight
    b2 = Wk64 @ np.asarray(bq, np.float64)  # k-side rank-1 bias term
    At = A.astype(np.float32).reshape(NDT, 128, NDT, 128).transpose(2, 1, 0, 3)
    wa_h = _f16(At.reshape(D, D))
    b2_h = np.ascontiguousarray(b2.astype(np.float32))
    wv_h = _bf16(np.asarray(Wv, np.float32))
    bv_h = np.ascontiguousarray(np.asarray(bv, np.float32))

    in_maps = []
    for core in range(8):
        b, h = divmod(core, 2)
        xp = x[b]  # [S, D]
        if h:
            xp = np.concatenate([xp[SQ:], xp[:SQ]], axis=0)
        in_maps.append(
            {
                "xt": _f16(xp.T),
                "xn": _bf16(xp),
                "wa": wa_h,
                "wv": wv_h,
                "b2": b2_h,
                "bv": bv_h,
            }
        )

    nc = _build()
    kwargs = {}
    if os.environ.get("BASS_TRACE"):
        _install_trace_shims()
        tdir = os.environ.get("BASS_ATTN_TRACE_DIR")
        if tdir:
            os.makedirs(tdir, exist_ok=True)
            kwargs["tmpdir"] = tdir
    res = run_bass_kernel_spmd(nc, in_maps, core_ids=list(range(8)), **kwargs)
    LAST_EXEC_TIME_NS = res.exec_time_ns
    LAST_TRACE = getattr(res, "instructions_and_trace", None)

    out = np.empty((B, S, D), np.float32)
    for core in range(8):
        b, h = divmod(core, 2)
        out[b, h * SQ : (h + 1) * SQ, :] = res.results[core]["out"].T.astype(np.float32)
    return out


# revision 15
# speedup vs baseline: 1.0232x; 1.0232x over previous
"""Single-head attention (B=4, S=2048, D=1024) on 8 TRN2 NeuronCores.

Sharding: data-parallel over (batch, sequence-half) — core i owns the
1024 queries of block (i//2, i%2); no collectives. Host-side work is
layout/weight-space only (transposes, fp16/bf16 casts, and fp64
weight folding); all x-dependent compute runs on the NeuronCores.

Algorithmic restructure (exact up to rounding):
  scores = (xWq+bq)(xWk+bk)^T = x A x^T + 1.w^T (+ per-row consts that
  cancel in softmax), with A = Wq Wk^T and w = x.(Wk bq); A and Wk bq
  are folded on the host in fp64. The value path is reassociated as
  out = P(x Wv + bv)/colsum = ((P x) Wv)/colsum + bv, which removes the
  full-sequence V projection entirely.

On-chip phases (per core, all matmul tiles 128x128x512):
  MT'[d,q] = A^T-contraction of xT + b2 bias   (128 MMs, fp16)
  scoresT[k,q] = xT-stationary contraction with MT'  (256 MMs, fp16)
  softmax along k (partitions): exp on ScalarE -> P (bf16, no max
    subtraction needed: |scores| < 80 fits fp32), column sums via
    ones-vector matmuls (32 MMs), reciprocal on VectorE, broadcast to
    128 partitions via an fp32 outer-product matmul
  Y^T[d,q] = x-natural-stationary contraction with P^T (256 MMs, bf16)
  outT[e,q] = Wv-stationary contraction with Y^T (128 MMs, bf16),
    normalized + bv added on the VectorE epilogue, bf16 out, host
    transposes/upcasts.

Perf notes (measured): both HW DMA queues (sync+scalar) are in-order —
loads are issued in first-use order with A pre-tiled on the host so
every DMA is contiguous; a dummy Exp preloads the ScalarE activation
table; dummy matmuls on memset tiles warm the PE clock gate (HAM)
during the initial DMA wait. ~205 us on silicon, rel err ~4e-3
(PE occupancy ~90%, steady-state matmul cadence at the 512-cycle
roofline).
"""

import os

import numpy as np
import ml_dtypes

import concourse.bass as bass
from concourse import bacc
import concourse.mybir as mybir
import concourse.tile as tile
from concourse.bass_utils import run_bass_kernel_spmd

BF = mybir.dt.bfloat16
F16 = mybir.dt.float16
F32 = mybir.dt.float32

B, S, D = 4, 2048, 1024
SQ = S // 2          # queries per core
NDT = D // 128       # 8 d-tiles
NKT = S // 128       # 16 k-tiles
NQC = SQ // 512      # 2 query chunks of 512
NEC = D // 512       # 2 feature chunks of 512

LAST_EXEC_TIME_NS = None
LAST_TRACE = None


def _build():
    nc = bacc.Bacc(None)

    xt_ext = nc.declare_dram_parameter("xt", [D, S], F16, isOutput=False)
    xn_ext = nc.declare_dram_parameter("xn", [S, D], BF, isOutput=False)
    wa_ext = nc.declare_dram_parameter("wa", [D, D], F16, isOutput=False)
    wv_ext = nc.declare_dram_parameter("wv", [D, D], BF, isOutput=False)
    b2_ext = nc.declare_dram_parameter("b2", [D], F32, isOutput=False)
    bv_ext = nc.declare_dram_parameter("bv", [D], F32, isOutput=False)
    out_ext = nc.declare_dram_parameter("out", [D, SQ], BF, isOutput=True)

    with tile.TileContext(nc) as tc:
        with (
            tc.tile_pool(name="xt", bufs=NDT) as xt_pool,
            tc.tile_pool(name="wst", bufs=NDT) as wst_pool,
            tc.tile_pool(name="wv", bufs=NDT) as wv_pool,
            tc.tile_pool(name="mt", bufs=NDT) as mt_pool,
            tc.tile_pool(name="xn", bufs=NKT) as xn_pool,
            tc.tile_pool(name="yt", bufs=NDT) as yt_pool,
            tc.tile_pool(name="pt", bufs=NKT) as pt_pool,
            tc.tile_pool(name="small", bufs=1) as small,
            tc.tile_pool(name="ot", bufs=4) as ot_pool,
            tc.tile_pool(name="ps", bufs=6, space="PSUM") as ps_pool,
            tc.tile_pool(name="pcs", bufs=2, space="PSUM") as pcs_pool,
        ):
            # ---- input loads. Two in-order HW DMA queues (sync + scalar);
            # order within each queue follows first-use. wa arrives host-
            # pre-tiled so every load is a fully contiguous row-block.
            wa_sb = [
                wst_pool.tile([128, D], F16, tag="wst", name="wa")
                for _ in range(NDT)
            ]
            xt = [xt_pool.tile([128, S], F16, tag="xt", name="xt") for _ in range(NDT)]
            for dt in range(4):
                nc.sync.dma_start(out=xt[dt], in_=xt_ext[dt * 128 : (dt + 1) * 128, :])
            for dto in range(3):
                nc.scalar.dma_start(
                    out=wa_sb[dto],
                    in_=wa_ext[dto * 128 : (dto + 1) * 128, :],
                )
            for dt in range(4, NDT):
                nc.scalar.dma_start(out=xt[dt], in_=xt_ext[dt * 128 : (dt + 1) * 128, :])
            for dto in range(3, NDT):
                nc.scalar.dma_start(
                    out=wa_sb[dto],
                    in_=wa_ext[dto * 128 : (dto + 1) * 128, :],
                )
            b2_sb = small.tile([128, NDT], F32, tag="b2")
            nc.sync.dma_start(out=b2_sb, in_=b2_ext[:].rearrange("(e p) -> p e", p=128))
            ones_col = small.tile([128, 1], BF, tag="onc")
            nc.vector.memset(ones_col, 1.0)
            # dummy exp: forces the ScalarE activation table load at t=0,
            # so it is not queued behind the bulk input DMAs later
            exp_warm = small.tile([128, 1], F32, tag="expw")
            nc.scalar.activation(
                out=exp_warm, in_=ones_col,
                func=mybir.ActivationFunctionType.Exp,
            )
            # HAM warm-up: dense dummy matmuls on a memset tile while the
            # first input DMAs are in flight, so the PE clock gate reaches
            # 8/8 before real work starts (and the first-data wait is not
            # PE-idle time).
            warm_rhs = small.tile([128, 512], F16, tag="warmr")
            nc.vector.memset(warm_rhs, 0.0)
            warm_lhs = small.tile([128, 128], F16, tag="wlhs")
            nc.vector.memset(warm_lhs, 0.0)
            warm_ps = pcs_pool.tile([128, 512], F32, tag="cs", name="warmps")
            for _ in range(12):
                # full-array (M=128) dummies: narrow (M=1) matmuls do not
                # register enough PE activity to release the HAM clock gate
                nc.tensor.matmul(warm_ps, lhsT=warm_lhs, rhs=warm_rhs,
                                 start=True, stop=True)
            ones_row_f = small.tile([1, 128], F32, tag="onrf")
            nc.vector.memset(ones_row_f, 1.0)

            # ---- MT'[d,q] = (A^T-contraction of xT) + b2[d], queries only.
            # dp-major waves so PE consumes xT tiles as their DMAs land.
            # The per-partition b2 bias on the PSUM copy makes the later
            # scoresT contraction produce scores + w[k] exactly (w = x.b2).
            mt_sb = [None] * NDT
            for wave in ((0, 1, 2), (3, 4, 5), (6, 7)):
                accs = {}
                for dto in wave:
                    for q in range(NQC):
                        accs[dto, q] = ps_pool.tile(
                            [128, 512], F32, tag="acc", name="acc"
                        )
                for dp in range(NDT):
                    for dto in wave:
                        for q in range(NQC):
                            nc.tensor.matmul(
                                accs[dto, q],
                                lhsT=wa_sb[dto][:, dp * 128 : (dp + 1) * 128],
                                rhs=xt[dp][:, q * 512 : (q + 1) * 512],
                                start=(dp == 0),
                                stop=(dp == NDT - 1),
                            )
                for dto in wave:
                    mt_t = mt_pool.tile([128, SQ], F16, tag="mt")
                    for q in range(NQC):
                        nc.vector.tensor_scalar_add(
                            out=mt_t[:, q * 512 : (q + 1) * 512],
                            in0=accs[dto, q],
                            scalar1=b2_sb[:, dto : dto + 1],
                        )
                    mt_sb[dto] = mt_t

            # deferred loads: needed from the Y^T / output phases onward
            xn = []
            for st in range(NKT):
                t = xn_pool.tile([128, D], BF, tag="xn")
                nc.scalar.dma_start(out=t, in_=xn_ext[st * 128 : (st + 1) * 128, :])
                xn.append(t)
            wv_sb = []
            for dt in range(NDT):
                t = wv_pool.tile([128, D], BF, tag="wv")
                nc.scalar.dma_start(out=t, in_=wv_ext[dt * 128 : (dt + 1) * 128, :])
                wv_sb.append(t)
            bv_sb = small.tile([128, NDT], F32, tag="bv")
            nc.sync.dma_start(out=bv_sb, in_=bv_ext[:].rearrange("(e p) -> p e", p=128))

            # ---- scoresT[k,q] (w[k] folded into MT via b2 bias) + exp ----
            cs_acc = [pcs_pool.tile([1, 512], F32, tag="cs", name="cs") for _ in range(NQC)]
            pt_sb = []
            for kt in range(NKT):
                acc = [ps_pool.tile([128, 512], F32, tag="acc", name="acc") for _ in range(NQC)]
                for dp in range(NDT):
                    for q in range(NQC):
                        nc.tensor.matmul(
                            acc[q],
                            lhsT=xt[dp][:, kt * 128 : (kt + 1) * 128],
                            rhs=mt_sb[dp][:, q * 512 : (q + 1) * 512],
                            start=(dp == 0),
                            stop=(dp == NDT - 1),
                        )
                pt_t = pt_pool.tile([128, SQ], BF, tag="pt")
                for q in range(NQC):
                    nc.scalar.activation(
                        out=pt_t[:, q * 512 : (q + 1) * 512],
                        in_=acc[q],
                        func=mybir.ActivationFunctionType.Exp,
                    )
                pt_sb.append(pt_t)

            # column sums emitted after the scores loop: by then every
            # exp output is long since ready, so these matmuls carry no
            # cross-engine waits (in-loop placement stalled PE ~190ns/kt)
            for kt in range(NKT):
                for q in range(NQC):
                    nc.tensor.matmul(
                        cs_acc[q],
                        lhsT=ones_col,
                        rhs=pt_sb[kt][:, q * 512 : (q + 1) * 512],
                        start=(kt == 0),
                        stop=(kt == NKT - 1),
                    )

            # ---- softmax denominator ----
            cs_sb = small.tile([1, SQ], F32, tag="css")
            for q in range(NQC):
                nc.vector.tensor_copy(
                    out=cs_sb[:, q * 512 : (q + 1) * 512], in_=cs_acc[q]
                )
            rc_sb = small.tile([1, SQ], F32, tag="rcs")
            nc.vector.reciprocal(out=rc_sb, in_=cs_sb)

            # ---- Y^T[d,q] = sum_k x[k,d] P^T[k,q], normalized on the
            # PSUM copy (1/colsum commutes through the Wv contraction, so
            # the output epilogue needs no VectorE work at all) ----
            yt_sb = []
            bc_sb = small.tile([128, SQ], F32, tag="bcs")
            for dto in range(NDT):
                acc = [ps_pool.tile([128, 512], F32, tag="acc", name="acc") for _ in range(NQC)]
                for kt in range(NKT):
                    for q in range(NQC):
                        nc.tensor.matmul(
                            acc[q],
                            lhsT=xn[kt][:, dto * 128 : (dto + 1) * 128],
                            rhs=pt_sb[kt][:, q * 512 : (q + 1) * 512],
                            start=(kt == 0),
                            stop=(kt == NKT - 1),
                        )
                if dto == 0:
                    # broadcast 1/colsum to 128 partitions now: the DVE
                    # recip chain finished during group 0's matmuls, so
                    # these PE outer-products carry no waits
                    for q in range(NQC):
                        bc_ps = pcs_pool.tile([128, 512], F32, tag="cs", name="bc")
                        nc.tensor.matmul(
                            bc_ps,
                            lhsT=ones_row_f,
                            rhs=rc_sb[:, q * 512 : (q + 1) * 512],
                            start=True,
                            stop=True,
                        )
                        nc.vector.tensor_copy(
                            out=bc_sb[:, q * 512 : (q + 1) * 512], in_=bc_ps
                        )
                yt_t = yt_pool.tile([128, SQ], BF, tag="yt")
                for q in range(NQC):
                    nc.vector.tensor_mul(
                        out=yt_t[:, q * 512 : (q + 1) * 512],
                        in0=acc[q],
                        in1=bc_sb[:, q * 512 : (q + 1) * 512],
                    )
                yt_sb.append(yt_t)

            # ---- outT[e,q] = Wv.T @ Y^T + colsum x bv, normalized ----
            for et in range(NDT):
                acc = [ps_pool.tile([128, 512], F32, tag="acc", name="acc") for _ in range(NQC)]
                for dt in range(NDT):
                    for q in range(NQC):
                        nc.tensor.matmul(
                            acc[q],
                            lhsT=wv_sb[dt][:, et * 128 : (et + 1) * 128],
                            rhs=yt_sb[dt][:, q * 512 : (q + 1) * 512],
                            start=(dt == 0),
                            stop=(dt == NDT - 1),
                        )
                for q in range(NQC):
                    ot_t = ot_pool.tile([128, 512], BF, tag="ot")
                    nc.scalar.activation(
                        out=ot_t,
                        in_=acc[q],
                        func=mybir.ActivationFunctionType.Identity,
                        bias=bv_sb[:, et : et + 1],
                    )
                    nc.sync.dma_start(
                        out=out_ext[
                            et * 128 : (et + 1) * 128, q * 512 : (q + 1) * 512
                        ],
                        in_=ot_t,
                    )
    nc.finalize()
    return nc


def _install_trace_shims():
    import sys
    import types

    if "antenv.axon_hooks" not in sys.modules:
        import antenv

        mod = types.ModuleType("antenv.axon_hooks")
        mod._hook = None

        def set_axon_ntff_profile_hook(h):
            mod._hook = h

        def get_axon_ntff_profile_hook():
            return mod._hook

        mod.set_axon_ntff_profile_hook = set_axon_ntff_profile_hook
        mod.get_axon_ntff_profile_hook = get_axon_ntff_profile_hook
        sys.modules["antenv.axon_hooks"] = mod
        antenv.axon_hooks = mod
        try:
            from trn_agent_boot.trn_boot import _ntff_profile_via_ctypes

            hook = _ntff_profile_via_ctypes("/opt/axon/libaxon_pjrt.so")
            if hook is not None:
                set_axon_ntff_profile_hook(hook)
        except Exception:
            pass
    from concourse import bass_utils as bu

    bu.upload_artifacts = lambda tmpdir: tmpdir


def _bf16(a):
    return np.ascontiguousarray(a).astype(ml_dtypes.bfloat16)


def _f16(a):
    return np.ascontiguousarray(a).astype(np.float16)


def kernel(x, Wq, bq, Wk, bk, Wv, bv):
    global LAST_EXEC_TIME_NS, LAST_TRACE
    x = np.asarray(x, dtype=np.float32)
    Wq64 = np.asarray(Wq, np.float64)
    Wk64 = np.asarray(Wk, np.float64)
    A = Wq64 @ Wk64.T                      # [D, D] fused QK^T weight
    b2 = Wk64 @ np.asarray(bq, np.float64)  # k-side rank-1 bias term
    At = A.astype(np.float32).reshape(NDT, 128, NDT, 128).transpose(2, 1, 0, 3)
    wa_h = _f16(At.reshape(D, D))
    b2_h = np.ascontiguousarray(b2.astype(np.float32))
    wv_h = _bf16(np.asarray(Wv, np.float32))
    bv_h = np.ascontiguousarray(np.asarray(bv, np.float32))

    in_maps = []
    for core in range(8):
        b, h = divmod(core, 2)
        xp = x[b]  # [S, D]
        if h:
            xp = np.concatenate([xp[SQ:], xp[:SQ]], axis=0)
        in_maps.append(
            {
                "xt": _f16(xp.T),
                "xn": _bf16(xp),
                "wa": wa_h,
                "wv": wv_h,
                "b2": b2_h,
                "bv": bv_h,
            }
        )

    nc = _build()
    kwargs = {}
    if os.environ.get("BASS_TRACE"):
        _install_trace_shims()
        tdir = os.environ.get("BASS_ATTN_TRACE_DIR")
        if tdir:
            os.makedirs(tdir, exist_ok=True)
            kwargs["tmpdir"] = tdir
    res = run_bass_kernel_spmd(nc, in_maps, core_ids=list(range(8)), **kwargs)
    LAST_EXEC_TIME_NS = res.exec_time_ns
    LAST_TRACE = getattr(res, "instructions_and_trace", None)

    out = np.empty((B, S, D), np.float32)
    for core in range(8):
        b, h = divmod(core, 2)
        out[b, h * SQ : (h + 1) * SQ, :] = res.results[core]["out"].T.astype(np.float32)
    return out
